# revision 13
# baseline (speedup 1.0000x reference)
"""EdgeDegreeEmbedding Trainium2 kernel (8 NeuronCores, SPMD, no collectives).

Strategy: shard by TARGET NODE (625 nodes/core). Host sorts edges by target
node and packs each node's first 16 edges into a 16-row "half"; four halves
(2 nodes x 2? no: 4 halves = 2 slots) -- a PAIR = 64 rows = 4 halves = 4
nodes' halves... Layout:
- a TILE is 128 edge rows = 8 halves = 8 nodes (16 rows each).
- rotation: per tile, two PAIRS of 64 rows each; for each m, one matmul per
  pair: stationary = m0[64 rows, 128 ch], moving = host-built 4-block-
  diagonal wigner slice [64, 196] (envelope/RESCALE folded in), accumulated
  over m in a per-pair [128,196] PSUM tile. The edge->node scatter-add
  happens inside the PE. 14 matmuls per tile instead of 28.
- tiles processed in GROUPS of 4: LayerNorm rsqrt (quake seed + 1 Newton
  iter) runs once per group on [128,4] batches; MLP psums are packed into
  [128,512] banks; h transposes are done by the DMA xbar (sync engine), not
  the PE; rotation of group g-1 fills the PE while group g's LN chains run.
- x is NOT read or added on device; the device writes bf16 messages in a
  partition-major layout [128, T*392]; the host adds x.

HW constraints learned by probing (crash NRT if violated):
- a matmul with tile_position (sub-128-row stationary) must write its PSUM
  tile at offset 0. Full-row matmuls may write at column offsets.
- a single matmul may not cross a 2KB PSUM bank boundary.
"""

import numpy as np

import concourse.bass as bass
import concourse.mybir as mybir
from concourse import tile
from concourse.bass_utils import run_bass_kernel_spmd
from concourse.vector_clock import ScopedClock

# ---- problem constants (hardcoded; must match the reference) ----
SPHERE = 128
M0 = 7
LFULL = 49
CUTOFF = 12.0
RESCALE = 23.395238876342773
LN_EPS = 1e-5
N_NODES, N_EDGES, D_DIST = 5000, 50000, 512

N_CORES = 8
NODES_PER_CORE = N_NODES // N_CORES  # 625
HALF = 16                 # edges per node-half (one node's main capacity)
NPT = 8                   # halves (nodes) per tile
TILE_E = HALF * NPT       # 128 edges per tile
GRP = 4                   # tiles per processing group
H_MAIN = 640              # 625 real nodes + 15 dummies -> T_MAIN = 80
T_MAIN = H_MAIN // NPT    # 80
WCOLS = M0 * 4 * LFULL    # 1372: 4-block-diagonal wigner section per row
XWF = 6 * 128 + WCOLS     # 768 + 1372 = 2140
OUTF = NPT * LFULL        # 392
RMAGIC = 0x5F3759DF
NEWTON_ITERS = 1

BF16 = mybir.dt.bfloat16
F32 = mybir.dt.float32
I32 = mybir.dt.int32
NP_BF16 = mybir.dt.np(BF16)

_CACHE = {}
TRACE = False      # set True (e.g. from test.py) to profile the run
TRACE_KW = {}      # extra kwargs for run_bass_kernel_spmd when tracing
LAST = None        # BassKernelResults of the most recent run


class _ChunkedDrainTC(tile.TileContext):
    """Walrus here rejects >1 sync wait per instruction; spread every
    multi-wait instruction's extras over preceding same-engine nops, and do
    the same for the Tile exit-drain's global-clock waits."""

    def _lower_ordered_insts(self, ordered):
        for bb_name, insts in ordered.items():
            out = []
            for inst in insts:
                si = getattr(inst, "sync_info", None)
                waits = list(si.on_wait) if si is not None and si.on_wait else []
                if len(waits) > 1 and type(inst).__name__.startswith("Inst"):
                    for w in waits[:-1]:
                        out.append(mybir.InstNoOp(
                            name=self.nc.get_next_instruction_name(),
                            sync_info=mybir.SyncInfo(on_wait=[w], on_update=[]),
                            bass_nofuse=True,
                            engine=inst.engine,
                        ))
                    si.on_wait = waits[-1:]
                out.append(inst)
            ordered[bb_name] = out
        return super()._lower_ordered_insts(ordered)

    def _drain_and_barrier(self, tick_clock, wait_clock):
        nc = self.nc
        probe = nc.sync.nop()
        wait_clock.add_sem_waits(
            probe.ins, ScopedClock({None: tick_clock.global_clock})
        )
        si = probe.ins.sync_info
        waits = list(si.on_wait) if si and si.on_wait else []
        si.on_wait = waits[:1]
        for w in waits[1:]:
            n2 = nc.sync.nop()
            n2.ins.sync_info = mybir.SyncInfo(on_wait=[w], on_update=[])
        nc.sync.drain()
        nc.all_engine_barrier()
        popped = nc._tile_sem_poison_stack.pop()
        assert popped is self._sem_poison
        nc.clear_and_free_semaphores(list(self.sems.allocated().values()))
        nc.all_engine_barrier()


def _build_nc(T):
    """Build the SPMD Bass program for T tiles (multiple of GRP)."""
    assert T % GRP == 0
    nc = bass.Bass("TRN2", target_bir_lowering=False, num_devices=N_CORES)

    xw = nc.dram_tensor("xw", [128, T * XWF], BF16, kind="ExternalInput")
    w1 = nc.dram_tensor("w1", [128, 6 * 128], BF16, kind="ExternalInput")
    w2 = nc.dram_tensor("w2", [128, 128], BF16, kind="ExternalInput")
    w3 = nc.dram_tensor("w3", [128, M0 * SPHERE], BF16, kind="ExternalInput")

    outr = nc.dram_tensor("outr", [128, T * OUTF], BF16, kind="ExternalOutput")

    AF = mybir.ActivationFunctionType
    ALU = mybir.AluOpType

    with _ChunkedDrainTC(nc) as tc:
        with (
            tc.tile_pool(name="const", bufs=1) as cpool,
            tc.tile_pool(name="xw", bufs=3) as xw_pool,
            tc.tile_pool(name="hs", bufs=2) as h_pool,
            tc.tile_pool(name="m0sb", bufs=10) as m0sb_pool,
            tc.tile_pool(name="outt", bufs=3) as out_pool,
            tc.tile_pool(name="stat", bufs=2) as stat_pool,
            tc.tile_pool(name="ps", bufs=2, space="PSUM") as ps_pool,
            tc.tile_pool(name="m0ps", bufs=2, space="PSUM") as m0ps_pool,
            tc.tile_pool(name="rot", bufs=2, space="PSUM") as rot_pool,
        ):
            w1_sb = cpool.tile([128, 6 * 128], BF16)
            nc.sync.dma_start(w1_sb[:], w1[:])
            w2_sb = cpool.tile([128, 128], BF16)
            nc.sync.dma_start(w2_sb[:], w2[:])
            w3_sb = cpool.tile([128, M0 * SPHERE], BF16)
            nc.sync.dma_start(w3_sb[:], w3[:])
            zero4 = cpool.tile([128, GRP], F32)
            nc.vector.memset(zero4[:], 0.0)

            def ln_chain(tagp):
                """Batched quake rsqrt for GRP tiles."""
                st = stat_pool.tile([128, 6 * GRP], F32, tag=f"st{tagp}")
                mv3 = stat_pool.tile([128, 2, GRP], F32, tag=f"mv{tagp}")
                ve = stat_pool.tile([128, GRP], F32, tag=f"ve{tagp}")
                yi = stat_pool.tile([128, GRP], I32, tag=f"yi{tagp}")
                t1 = stat_pool.tile([128, GRP], F32, tag=f"t1{tagp}")
                nm = stat_pool.tile([128, GRP], F32, tag=f"nm{tagp}")
                yf = yi[:].bitcast(F32)

                def finish():
                    var, mu = mv3[:, 1, :], mv3[:, 0, :]
                    nc.vector.tensor_scalar(ve[:], var, LN_EPS, None, ALU.add)
                    nc.vector.tensor_scalar(yi[:], ve[:].bitcast(I32), 1, None,
                                            ALU.arith_shift_right)
                    nc.vector.tensor_scalar(yi[:], yi[:], -1, RMAGIC,
                                            ALU.mult, ALU.add)
                    for _ in range(NEWTON_ITERS):
                        nc.gpsimd.tensor_mul(t1[:], yf, yf)
                        nc.gpsimd.tensor_mul(t1[:], t1[:], ve[:])
                        nc.vector.tensor_scalar(t1[:], t1[:], -0.5, 1.5,
                                                ALU.mult, ALU.add)
                        nc.gpsimd.tensor_mul(yf, yf, t1[:])
                    nc.gpsimd.tensor_mul(nm[:], mu, yf)
                    nc.gpsimd.tensor_sub(nm[:], zero4[:], nm[:])

                return st, mv3, yf, nm, finish

            def rot_phase(p):
                """Rotation for one previous tile: 2 pair-matmul chains.
                og is the group output sbuf tile; j its slot there."""
                xwg, base, og, j, m0_sb = p
                rts = [rot_pool.tile([128, 196], F32, tag="rot",
                                     name=f"rot{h}") for h in range(2)]
                for m in range(M0):
                    for pair, r in enumerate(rts):
                        pb = 64 * pair
                        nc.tensor.matmul(
                            r[:],
                            m0_sb[pb:pb + 64, m * 128:(m + 1) * 128],
                            xwg[pb:pb + 64,
                                base + 768 + m * 196:base + 768 + (m + 1) * 196],
                            start=(m == 0), stop=(m == M0 - 1),
                            tile_position=(pb, 0),
                        )
                for pair, r in enumerate(rts):
                    nc.scalar.activation(
                        og[:, j * OUTF + pair * 196:j * OUTF + (pair + 1) * 196],
                        r[:], AF.Copy)

            prev = []
            prev_og = None
            for g in range(T // GRP):
                base0 = g * GRP * XWF

                xwg = xw_pool.tile([128, GRP * XWF], BF16)
                nc.sync.dma_start(xwg[:], xw[:, base0:base0 + GRP * XWF])
                og = out_pool.tile([128, GRP * OUTF], BF16)

                # ---- section A: MLP layer 1 + stats ----
                ps1 = ps_pool.tile([128, 512], F32, tag="ps")
                st1, mv1, yf1, nm1, fin1 = ln_chain("a")
                for j in range(GRP):
                    base = j * XWF
                    for k in range(6):
                        nc.tensor.matmul(
                            ps1[:, j * 128:(j + 1) * 128],
                            xwg[:, base + k * 128:base + (k + 1) * 128],
                            w1_sb[:, k * 128:(k + 1) * 128],
                            start=(k == 0), stop=(k == 5),
                        )
                    nc.vector.bn_stats(st1[:, 6 * j:6 * j + 6],
                                       ps1[:, j * 128:(j + 1) * 128])
                    nc.vector.bn_aggr(mv1[:, :, j], st1[:, 6 * j:6 * j + 6])
                    if j == 1 and prev:
                        rot_phase(prev[0])
                        rot_phase(prev[1])
                fin1()

                # ---- section C: silu1, dma-transpose, MLP layer 2 ----
                h1_all = h_pool.tile([128, 512], BF16, tag="h1")
                for j in range(GRP):
                    nc.scalar.activation(h1_all[:, j * 128:(j + 1) * 128],
                                         ps1[:, j * 128:(j + 1) * 128],
                                         AF.Silu,
                                         bias=nm1[:, j:j + 1],
                                         scale=yf1[:, j:j + 1])
                h1t_all = h_pool.tile([128, 512], BF16, tag="h1t")
                for j in range(GRP):
                    nc.sync.dma_start_transpose(
                        h1t_all[:, j * 128:(j + 1) * 128],
                        h1_all[:, j * 128:(j + 1) * 128])

                ps2 = ps_pool.tile([128, 512], F32, tag="ps")
                st2, mv2, yf2, nm2, fin2 = ln_chain("b")
                for j in range(GRP):
                    nc.tensor.matmul(ps2[:, j * 128:(j + 1) * 128],
                                     h1t_all[:, j * 128:(j + 1) * 128],
                                     w2_sb[:], start=True, stop=True)
                    nc.vector.bn_stats(st2[:, 6 * j:6 * j + 6],
                                       ps2[:, j * 128:(j + 1) * 128])
                    nc.vector.bn_aggr(mv2[:, :, j], st2[:, 6 * j:6 * j + 6])
                    if j == 1 and prev:
                        rot_phase(prev[2])
                        rot_phase(prev[3])
                fin2()
                if prev:
                    nc.gpsimd.dma_start(
                        outr[:, (g - 1) * GRP * OUTF:g * GRP * OUTF],
                        prev_og[:])

                # ---- section E: silu2, dma-transpose, MLP layer 3 ----
                h2_all = h_pool.tile([128, 512], BF16, tag="h2")
                for j in range(GRP):
                    nc.scalar.activation(h2_all[:, j * 128:(j + 1) * 128],
                                         ps2[:, j * 128:(j + 1) * 128],
                                         AF.Silu,
                                         bias=nm2[:, j:j + 1],
                                         scale=yf2[:, j:j + 1])
                h2t_all = h_pool.tile([128, 512], BF16, tag="h2t")
                prev = []
                for j in range(GRP):
                    nc.sync.dma_start_transpose(
                        h2t_all[:, j * 128:(j + 1) * 128],
                        h2_all[:, j * 128:(j + 1) * 128])
                    # [128,1024] so each 448-col matmul stays inside a bank
                    m0ps = m0ps_pool.tile([128, 1024], F32, tag="m0")
                    nc.tensor.matmul(m0ps[:, 0:448],
                                     h2t_all[:, j * 128:(j + 1) * 128],
                                     w3_sb[:, 0:448], start=True, stop=True)
                    nc.tensor.matmul(m0ps[:, 512:960],
                                     h2t_all[:, j * 128:(j + 1) * 128],
                                     w3_sb[:, 448:896], start=True, stop=True)
                    m0_sb = m0sb_pool.tile([128, M0 * SPHERE], BF16)
                    nc.scalar.activation(m0_sb[:, 0:448], m0ps[:, 0:448],
                                         AF.Copy)
                    nc.vector.tensor_copy(m0_sb[:, 448:896], m0ps[:, 512:960])
                    prev.append((xwg, j * XWF, og, j, m0_sb))
                prev_og = og

            for p in prev:
                rot_phase(p)
            nc.gpsimd.dma_start(outr[:, (T - GRP) * OUTF:T * OUTF], prev_og[:])

    return nc


def _envelope(d):
    e = 1.0 + (-21.0) * d ** 5 + 35.0 * d ** 6 + (-15.0) * d ** 7
    return np.where(d < 1.0, e, 0.0)


def kernel(**inputs):
    x = np.asarray(inputs["x"], np.float32)
    dist_emb = np.asarray(inputs["edge_distance_embedding"], np.float32)
    src_emb = np.asarray(inputs["source_atom_embedding"], np.float32)
    tgt_emb = np.asarray(inputs["target_atom_embedding"], np.float32)
    edge_distance = np.asarray(inputs["edge_distance"], np.float64)
    edge_index = np.asarray(inputs["edge_index"]).astype(np.int64)
    wigner = np.asarray(inputs["wigner_and_M_mapping_inv"], np.float32)
    W1 = np.asarray(inputs["W1"], np.float32)
    W2 = np.asarray(inputs["W2"], np.float32)
    W3 = np.asarray(inputs["W3"], np.float32)
    # biases/gains are zeros/ones by construction; folded out of the kernel
    for nm, triv in (("b1", 0), ("bt1", 0), ("b2", 0), ("bt2", 0), ("b3", 0),
                     ("g1", 1), ("g2", 1)):
        v = np.asarray(inputs[nm])
        assert np.all(v == triv), f"{nm} not trivial; unsupported fast path"

    srcs, tgts = edge_index[0], edge_index[1]
    scale = (_envelope(edge_distance / CUTOFF) / RESCALE).astype(np.float32)

    order = np.argsort(tgts, kind="stable")
    tsorted = tgts[order]
    starts = np.searchsorted(tsorted, np.arange(N_NODES + 1))

    # ---- build halves per core (a half = <=16 edges of one node) ----
    core_halves = []
    max_ov = 0
    for c in range(N_CORES):
        halves_main = []
        halves_ov = []
        base = c * NODES_PER_CORE
        for nl in range(NODES_PER_CORE):
            eids = order[starts[base + nl]:starts[base + nl + 1]]
            halves_main.append((nl, eids[:HALF]))
            rest = eids[HALF:]
            while len(rest) > 0:
                halves_ov.append((nl, rest[:HALF]))
                rest = rest[HALF:]
        for nl in range(NODES_PER_CORE, H_MAIN):
            halves_main.append((nl, np.empty(0, np.int64)))  # dummy
        core_halves.append((halves_main, halves_ov))
        max_ov = max(max_ov, len(halves_ov))

    # pad overflow so T is a multiple of GRP (32 halves = 4 tiles)
    H_OV = max(GRP * NPT, -(-max_ov // (GRP * NPT)) * (GRP * NPT))
    H = H_MAIN + H_OV
    T = H // NPT
    E_pad = H * HALF

    if T not in _CACHE:
        _CACHE[T] = _build_nc(T)
    nc = _CACHE[T]

    # ---- shared weight tensors ----
    w1_in = np.ascontiguousarray(
        W1.reshape(6, 128, 128).transpose(1, 0, 2).reshape(128, 6 * 128)
    ).astype(NP_BF16)
    w2_in = W2.astype(NP_BF16)
    w3_in = W3.astype(NP_BF16)

    in_maps = []
    ov_maps = []
    for c in range(N_CORES):
        halves_main, halves_ov = core_halves[c]
        halves = halves_main + halves_ov + [
            (0, np.empty(0, np.int64))
        ] * (H_OV - len(halves_ov))

        eorder = np.full(E_pad, -1, np.int64)
        for s, (_, eids) in enumerate(halves):
            eorder[s * HALF:s * HALF + len(eids)] = eids
        valid = eorder >= 0
        idx = eorder[valid]

        # xe gather -> [E_pad, 768] -> [T, 128p(c), 6k x 128e]
        xe = np.zeros((E_pad, 768), np.float32)
        xe[valid, :D_DIST] = dist_emb[idx]
        xe[valid, D_DIST:D_DIST + 128] = src_emb[srcs[idx]]
        xe[valid, D_DIST + 128:] = tgt_emb[tgts[idx]]
        xeT = xe.reshape(T, TILE_E, 6, 128).transpose(0, 3, 2, 1)

        # 4-block-diagonal wigner section per 64-row pair:
        # xw[t, 64p+16q+i, 768 + m*196 + q*49 + f] = wig[e,f,m]*scale
        wrows = np.zeros((E_pad, M0, LFULL), np.float32)
        wrows[valid] = (
            wigner[idx, :, :M0] * scale[idx][:, None, None]
        ).transpose(0, 2, 1)
        wr5 = wrows.reshape(T, 2, 4, HALF, M0, LFULL)
        wsec = np.zeros((T, 2, 4, HALF, M0, 4, LFULL), np.float32)
        for q in range(4):
            wsec[:, :, q, :, :, q, :] = wr5[:, :, q]
        wsec = wsec.reshape(T, 128, WCOLS)

        # partition-major layout: [128, T*XWF]
        xw_in = np.ascontiguousarray(np.concatenate(
            (xeT.reshape(T, 128, 768), wsec), axis=2,
        ).transpose(1, 0, 2).reshape(128, T * XWF)).astype(NP_BF16)

        in_maps.append({
            "xw": xw_in, "w1": w1_in, "w2": w2_in, "w3": w3_in,
        })
        ov_maps.append([nl for nl, _ in halves_ov])

    global LAST
    res = run_bass_kernel_spmd(
        nc, in_maps, core_ids=list(range(N_CORES)), trace=TRACE, **TRACE_KW
    )
    LAST = res

    out = x.copy()
    for c in range(N_CORES):
        r = res.results[c]
        # [128c, T, 8, 49] -> [H, 49, 128]
        o = np.asarray(r["outr"]).astype(np.float32).reshape(
            128, T, NPT, LFULL).transpose(1, 2, 3, 0).reshape(
            H, LFULL, 128)
        oc = o[:NODES_PER_CORE]
        for s, nl in enumerate(ov_maps[c]):
            oc[nl] += o[H_MAIN + s]
        out[c * NODES_PER_CORE:(c + 1) * NODES_PER_CORE] += oc
    return out


# revision 14
# speedup vs baseline: 1.3422x; 1.3422x over previous
"""EdgeDegreeEmbedding Trainium2 kernel (8 NeuronCores, SPMD, no collectives).

Strategy: shard by TARGET NODE (625 nodes/core). Host sorts edges by target
node and packs each node's first 16 edges into a 16-row "half"; four halves
(2 nodes x 2? no: 4 halves = 2 slots) -- a PAIR = 64 rows = 4 halves = 4
nodes' halves... Layout:
- a TILE is 128 edge rows = 8 halves = 8 nodes (16 rows each).
- rotation: per tile, two PAIRS of 64 rows each; for each m, one matmul per
  pair: stationary = m0[64 rows, 128 ch], moving = host-built 4-block-
  diagonal wigner slice [64, 196] (envelope/RESCALE folded in), accumulated
  over m in a per-pair [128,196] PSUM tile. The edge->node scatter-add
  happens inside the PE. 14 matmuls per tile instead of 28.
- tiles processed in GROUPS of 4: LayerNorm rsqrt (quake seed + 1 Newton
  iter) runs once per group on [128,4] batches; MLP psums are packed into
  [128,512] banks; h transposes are done by the DMA xbar (sync engine), not
  the PE; rotation of group g-1 fills the PE while group g's LN chains run.
- x is NOT read or added on device; the device writes bf16 messages in a
  partition-major layout [128, T*392]; the host adds x.

HW constraints learned by probing (crash NRT if violated):
- a matmul with tile_position (sub-128-row stationary) must write its PSUM
  tile at offset 0. Full-row matmuls may write at column offsets.
- a single matmul may not cross a 2KB PSUM bank boundary.
"""

import numpy as np

import concourse.bass as bass
import concourse.mybir as mybir
from concourse import tile
from concourse.bass_utils import run_bass_kernel_spmd
from concourse.vector_clock import ScopedClock

# ---- problem constants (hardcoded; must match the reference) ----
SPHERE = 128
M0 = 7
LFULL = 49
CUTOFF = 12.0
RESCALE = 23.395238876342773
LN_EPS = 1e-5
N_NODES, N_EDGES, D_DIST = 5000, 50000, 512

N_CORES = 8
NODES_PER_CORE = N_NODES // N_CORES  # 625
HALF = 16                 # edges per node-half (one node's main capacity)
NPT = 8                   # halves (nodes) per tile
TILE_E = HALF * NPT       # 128 edges per tile
GRP = 4                   # tiles per processing group
H_MAIN = 640              # 625 real nodes + 15 dummies -> T_MAIN = 80
T_MAIN = H_MAIN // NPT    # 80
WCOLS = M0 * 4 * LFULL    # 1372: 4-block-diagonal wigner section per row
XWF = 6 * 128 + WCOLS     # 768 + 1372 = 2140
OUTF = NPT * LFULL        # 392
RMAGIC = 0x5F3759DF
NEWTON_ITERS = 1

BF16 = mybir.dt.bfloat16
F32 = mybir.dt.float32
I32 = mybir.dt.int32
NP_BF16 = mybir.dt.np(BF16)

_CACHE = {}
TRACE = False      # set True (e.g. from test.py) to profile the run
TRACE_KW = {}      # extra kwargs for run_bass_kernel_spmd when tracing
LAST = None        # BassKernelResults of the most recent run


class _ChunkedDrainTC(tile.TileContext):
    """Walrus here rejects >1 sync wait per instruction; spread every
    multi-wait instruction's extras over preceding same-engine nops, and do
    the same for the Tile exit-drain's global-clock waits."""

    def _lower_ordered_insts(self, ordered):
        for bb_name, insts in ordered.items():
            out = []
            for inst in insts:
                si = getattr(inst, "sync_info", None)
                waits = list(si.on_wait) if si is not None and si.on_wait else []
                if len(waits) > 1 and type(inst).__name__.startswith("Inst"):
                    for w in waits[:-1]:
                        out.append(mybir.InstNoOp(
                            name=self.nc.get_next_instruction_name(),
                            sync_info=mybir.SyncInfo(on_wait=[w], on_update=[]),
                            bass_nofuse=True,
                            engine=inst.engine,
                        ))
                    si.on_wait = waits[-1:]
                out.append(inst)
            ordered[bb_name] = out
        return super()._lower_ordered_insts(ordered)

    def _drain_and_barrier(self, tick_clock, wait_clock):
        nc = self.nc
        probe = nc.sync.nop()
        wait_clock.add_sem_waits(
            probe.ins, ScopedClock({None: tick_clock.global_clock})
        )
        si = probe.ins.sync_info
        waits = list(si.on_wait) if si and si.on_wait else []
        si.on_wait = waits[:1]
        for w in waits[1:]:
            n2 = nc.sync.nop()
            n2.ins.sync_info = mybir.SyncInfo(on_wait=[w], on_update=[])
        nc.sync.drain()
        nc.all_engine_barrier()
        popped = nc._tile_sem_poison_stack.pop()
        assert popped is self._sem_poison
        nc.clear_and_free_semaphores(list(self.sems.allocated().values()))
        nc.all_engine_barrier()


def _build_nc(T):
    """Build the SPMD Bass program for T tiles (multiple of GRP)."""
    assert T % GRP == 0
    nc = bass.Bass("TRN2", target_bir_lowering=False, num_devices=N_CORES)

    xw = nc.dram_tensor("xw", [128, T * XWF], BF16, kind="ExternalInput")
    w1 = nc.dram_tensor("w1", [128, 6 * 128], BF16, kind="ExternalInput")
    w2 = nc.dram_tensor("w2", [128, 128], BF16, kind="ExternalInput")
    w3 = nc.dram_tensor("w3", [128, M0 * SPHERE], BF16, kind="ExternalInput")

    ident = nc.dram_tensor("ident", [128, 128], BF16, kind="ExternalInput")
    outr = nc.dram_tensor("outr", [128, T * OUTF], BF16, kind="ExternalOutput")

    AF = mybir.ActivationFunctionType
    ALU = mybir.AluOpType

    with _ChunkedDrainTC(nc) as tc:
        with (
            tc.tile_pool(name="const", bufs=1) as cpool,
            tc.tile_pool(name="xw", bufs=3) as xw_pool,
            tc.tile_pool(name="hs", bufs=2) as h_pool,
            tc.tile_pool(name="m0sb", bufs=10) as m0sb_pool,
            tc.tile_pool(name="outt", bufs=3) as out_pool,
            tc.tile_pool(name="stat", bufs=2) as stat_pool,
            tc.tile_pool(name="ps", bufs=2, space="PSUM") as ps_pool,
            tc.tile_pool(name="pst", bufs=2, space="PSUM") as pst_pool,
            tc.tile_pool(name="m0ps", bufs=1, space="PSUM") as m0ps_pool,
            tc.tile_pool(name="rot", bufs=2, space="PSUM") as rot_pool,
        ):
            w1_sb = cpool.tile([128, 6 * 128], BF16)
            nc.sync.dma_start(w1_sb[:], w1[:])
            w2_sb = cpool.tile([128, 128], BF16)
            nc.sync.dma_start(w2_sb[:], w2[:])
            w3_sb = cpool.tile([128, M0 * SPHERE], BF16)
            nc.sync.dma_start(w3_sb[:], w3[:])
            id_sb = cpool.tile([128, 128], BF16)
            nc.sync.dma_start(id_sb[:], ident[:])
            zero4 = cpool.tile([128, GRP], F32)
            nc.vector.memset(zero4[:], 0.0)

            def ln_chain(tagp):
                """Batched quake rsqrt for GRP tiles."""
                st = stat_pool.tile([128, 6 * GRP], F32, tag=f"st{tagp}")
                mv3 = stat_pool.tile([128, 2, GRP], F32, tag=f"mv{tagp}")
                ve = stat_pool.tile([128, GRP], F32, tag=f"ve{tagp}")
                yi = stat_pool.tile([128, GRP], I32, tag=f"yi{tagp}")
                t1 = stat_pool.tile([128, GRP], F32, tag=f"t1{tagp}")
                nm = stat_pool.tile([128, GRP], F32, tag=f"nm{tagp}")
                yf = yi[:].bitcast(F32)

                def finish():
                    var, mu = mv3[:, 1, :], mv3[:, 0, :]
                    nc.vector.tensor_scalar(ve[:], var, LN_EPS, None, ALU.add)
                    nc.vector.tensor_scalar(yi[:], ve[:].bitcast(I32), 1, None,
                                            ALU.arith_shift_right)
                    nc.vector.tensor_scalar(yi[:], yi[:], -1, RMAGIC,
                                            ALU.mult, ALU.add)
                    for _ in range(NEWTON_ITERS):
                        nc.gpsimd.tensor_mul(t1[:], yf, yf)
                        nc.gpsimd.tensor_mul(t1[:], t1[:], ve[:])
                        nc.vector.tensor_scalar(t1[:], t1[:], -0.5, 1.5,
                                                ALU.mult, ALU.add)
                        nc.gpsimd.tensor_mul(yf, yf, t1[:])
                    nc.gpsimd.tensor_mul(nm[:], mu, yf)
                    nc.gpsimd.tensor_sub(nm[:], zero4[:], nm[:])

                return st, mv3, yf, nm, finish

            def rot_phase(p):
                """Rotation for one previous tile: 2 pair-matmul chains.
                og is the group output sbuf tile; j its slot there."""
                xwg, base, og, j, m0_sb = p
                rts = [rot_pool.tile([128, 196], F32, tag="rot",
                                     name=f"rot{h}") for h in range(2)]
                for m in range(M0):
                    for pair, r in enumerate(rts):
                        pb = 64 * pair
                        nc.tensor.matmul(
                            r[:],
                            m0_sb[pb:pb + 64, m * 128:(m + 1) * 128],
                            xwg[pb:pb + 64,
                                base + 768 + m * 196:base + 768 + (m + 1) * 196],
                            start=(m == 0), stop=(m == M0 - 1),
                            tile_position=(pb, 0),
                        )
                for pair, r in enumerate(rts):
                    nc.scalar.activation(
                        og[:, j * OUTF + pair * 196:j * OUTF + (pair + 1) * 196],
                        r[:], AF.Copy)

            prev = []
            prev_og = None
            for g in range(T // GRP):
                base0 = g * GRP * XWF

                xwg = xw_pool.tile([128, GRP * XWF], BF16)
                nc.sync.dma_start(xwg[:], xw[:, base0:base0 + GRP * XWF])
                og = out_pool.tile([128, GRP * OUTF], BF16)

                # ---- section A: MLP layer 1 + stats ----
                ps1 = ps_pool.tile([128, 512], F32, tag="ps")
                st1, mv1, yf1, nm1, fin1 = ln_chain("a")
                for j in range(GRP):
                    base = j * XWF
                    for k in range(6):
                        nc.tensor.matmul(
                            ps1[:, j * 128:(j + 1) * 128],
                            xwg[:, base + k * 128:base + (k + 1) * 128],
                            w1_sb[:, k * 128:(k + 1) * 128],
                            start=(k == 0), stop=(k == 5),
                        )
                    nc.vector.bn_stats(st1[:, 6 * j:6 * j + 6],
                                       ps1[:, j * 128:(j + 1) * 128])
                    nc.vector.bn_aggr(mv1[:, :, j], st1[:, 6 * j:6 * j + 6])
                    if j == 1 and prev:
                        rot_phase(prev[0])
                        rot_phase(prev[1])
                fin1()

                # ---- section C: silu1, dma-transpose, MLP layer 2 ----
                h1_all = h_pool.tile([128, 512], BF16, tag="h1")
                for j in range(GRP):
                    nc.scalar.activation(h1_all[:, j * 128:(j + 1) * 128],
                                         ps1[:, j * 128:(j + 1) * 128],
                                         AF.Silu,
                                         bias=nm1[:, j:j + 1],
                                         scale=yf1[:, j:j + 1])
                pst1 = pst_pool.tile([128, 512], BF16, tag="pst")
                for j in range(GRP):
                    nc.tensor.transpose(pst1[:, j * 128:(j + 1) * 128],
                                        h1_all[:, j * 128:(j + 1) * 128],
                                        id_sb[:])
                h1t_all = h_pool.tile([128, 512], BF16, tag="h1t")
                nc.vector.tensor_copy(h1t_all[:], pst1[:])

                ps2 = ps_pool.tile([128, 512], F32, tag="ps")
                st2, mv2, yf2, nm2, fin2 = ln_chain("b")
                for j in range(GRP):
                    nc.tensor.matmul(ps2[:, j * 128:(j + 1) * 128],
                                     h1t_all[:, j * 128:(j + 1) * 128],
                                     w2_sb[:], start=True, stop=True)
                    nc.vector.bn_stats(st2[:, 6 * j:6 * j + 6],
                                       ps2[:, j * 128:(j + 1) * 128])
                    nc.vector.bn_aggr(mv2[:, :, j], st2[:, 6 * j:6 * j + 6])
                    if j == 1 and prev:
                        rot_phase(prev[2])
                        rot_phase(prev[3])
                fin2()
                if prev:
                    nc.gpsimd.dma_start(
                        outr[:, (g - 1) * GRP * OUTF:g * GRP * OUTF],
                        prev_og[:])

                # ---- section E: silu2, dma-transpose, MLP layer 3 ----
                h2_all = h_pool.tile([128, 512], BF16, tag="h2")
                for j in range(GRP):
                    nc.scalar.activation(h2_all[:, j * 128:(j + 1) * 128],
                                         ps2[:, j * 128:(j + 1) * 128],
                                         AF.Silu,
                                         bias=nm2[:, j:j + 1],
                                         scale=yf2[:, j:j + 1])
                h2t_all = h_pool.tile([128, 512], BF16, tag="h2t")
                pst2 = pst_pool.tile([128, 512], BF16, tag="pst")
                prev = []
                for j in range(GRP):
                    nc.tensor.transpose(pst2[:, j * 128:(j + 1) * 128],
                                        h2_all[:, j * 128:(j + 1) * 128],
                                        id_sb[:])
                    nc.vector.tensor_copy(h2t_all[:, j * 128:(j + 1) * 128],
                                          pst2[:, j * 128:(j + 1) * 128])
                    # [128,1024] so each 448-col matmul stays inside a bank
                    m0ps = m0ps_pool.tile([128, 1024], F32, tag="m0")
                    nc.tensor.matmul(m0ps[:, 0:448],
                                     h2t_all[:, j * 128:(j + 1) * 128],
                                     w3_sb[:, 0:448], start=True, stop=True)
                    nc.tensor.matmul(m0ps[:, 512:960],
                                     h2t_all[:, j * 128:(j + 1) * 128],
                                     w3_sb[:, 448:896], start=True, stop=True)
                    m0_sb = m0sb_pool.tile([128, M0 * SPHERE], BF16)
                    nc.scalar.activation(m0_sb[:, 0:448], m0ps[:, 0:448],
                                         AF.Copy)
                    nc.vector.tensor_copy(m0_sb[:, 448:896], m0ps[:, 512:960])
                    prev.append((xwg, j * XWF, og, j, m0_sb))
                prev_og = og

            for p in prev:
                rot_phase(p)
            nc.gpsimd.dma_start(outr[:, (T - GRP) * OUTF:T * OUTF], prev_og[:])

    return nc


def _envelope(d):
    e = 1.0 + (-21.0) * d ** 5 + 35.0 * d ** 6 + (-15.0) * d ** 7
    return np.where(d < 1.0, e, 0.0)


def kernel(**inputs):
    x = np.asarray(inputs["x"], np.float32)
    dist_emb = np.asarray(inputs["edge_distance_embedding"], np.float32)
    src_emb = np.asarray(inputs["source_atom_embedding"], np.float32)
    tgt_emb = np.asarray(inputs["target_atom_embedding"], np.float32)
    edge_distance = np.asarray(inputs["edge_distance"], np.float64)
    edge_index = np.asarray(inputs["edge_index"]).astype(np.int64)
    wigner = np.asarray(inputs["wigner_and_M_mapping_inv"], np.float32)
    W1 = np.asarray(inputs["W1"], np.float32)
    W2 = np.asarray(inputs["W2"], np.float32)
    W3 = np.asarray(inputs["W3"], np.float32)
    # biases/gains are zeros/ones by construction; folded out of the kernel
    for nm, triv in (("b1", 0), ("bt1", 0), ("b2", 0), ("bt2", 0), ("b3", 0),
                     ("g1", 1), ("g2", 1)):
        v = np.asarray(inputs[nm])
        assert np.all(v == triv), f"{nm} not trivial; unsupported fast path"

    srcs, tgts = edge_index[0], edge_index[1]
    scale = (_envelope(edge_distance / CUTOFF) / RESCALE).astype(np.float32)

    order = np.argsort(tgts, kind="stable")
    tsorted = tgts[order]
    starts = np.searchsorted(tsorted, np.arange(N_NODES + 1))

    # ---- build halves per core (a half = <=16 edges of one node) ----
    core_halves = []
    max_ov = 0
    for c in range(N_CORES):
        halves_main = []
        halves_ov = []
        base = c * NODES_PER_CORE
        for nl in range(NODES_PER_CORE):
            eids = order[starts[base + nl]:starts[base + nl + 1]]
            halves_main.append((nl, eids[:HALF]))
            rest = eids[HALF:]
            while len(rest) > 0:
                halves_ov.append((nl, rest[:HALF]))
                rest = rest[HALF:]
        for nl in range(NODES_PER_CORE, H_MAIN):
            halves_main.append((nl, np.empty(0, np.int64)))  # dummy
        core_halves.append((halves_main, halves_ov))
        max_ov = max(max_ov, len(halves_ov))

    # pad overflow so T is a multiple of GRP (32 halves = 4 tiles)
    H_OV = max(GRP * NPT, -(-max_ov // (GRP * NPT)) * (GRP * NPT))
    H = H_MAIN + H_OV
    T = H // NPT
    E_pad = H * HALF

    if T not in _CACHE:
        _CACHE[T] = _build_nc(T)
    nc = _CACHE[T]

    # ---- shared weight tensors ----
    w1_in = np.ascontiguousarray(
        W1.reshape(6, 128, 128).transpose(1, 0, 2).reshape(128, 6 * 128)
    ).astype(NP_BF16)
    w2_in = W2.astype(NP_BF16)
    w3_in = W3.astype(NP_BF16)
    ident = np.eye(128, dtype=np.float32).astype(NP_BF16)

    in_maps = []
    ov_maps = []
    for c in range(N_CORES):
        halves_main, halves_ov = core_halves[c]
        halves = halves_main + halves_ov + [
            (0, np.empty(0, np.int64))
        ] * (H_OV - len(halves_ov))

        eorder = np.full(E_pad, -1, np.int64)
        for s, (_, eids) in enumerate(halves):
            eorder[s * HALF:s * HALF + len(eids)] = eids
        valid = eorder >= 0
        idx = eorder[valid]

        # xe gather -> [E_pad, 768] -> [T, 128p(c), 6k x 128e]
        xe = np.zeros((E_pad, 768), np.float32)
        xe[valid, :D_DIST] = dist_emb[idx]
        xe[valid, D_DIST:D_DIST + 128] = src_emb[srcs[idx]]
        xe[valid, D_DIST + 128:] = tgt_emb[tgts[idx]]
        xeT = xe.reshape(T, TILE_E, 6, 128).transpose(0, 3, 2, 1)

        # 4-block-diagonal wigner section per 64-row pair:
        # xw[t, 64p+16q+i, 768 + m*196 + q*49 + f] = wig[e,f,m]*scale
        wrows = np.zeros((E_pad, M0, LFULL), np.float32)
        wrows[valid] = (
            wigner[idx, :, :M0] * scale[idx][:, None, None]
        ).transpose(0, 2, 1)
        wr5 = wrows.reshape(T, 2, 4, HALF, M0, LFULL)
        wsec = np.zeros((T, 2, 4, HALF, M0, 4, LFULL), np.float32)
        for q in range(4):
            wsec[:, :, q, :, :, q, :] = wr5[:, :, q]
        wsec = wsec.reshape(T, 128, WCOLS)

        # partition-major layout: [128, T*XWF]
        xw_in = np.ascontiguousarray(np.concatenate(
            (xeT.reshape(T, 128, 768), wsec), axis=2,
        ).transpose(1, 0, 2).reshape(128, T * XWF)).astype(NP_BF16)

        in_maps.append({
            "xw": xw_in, "w1": w1_in, "w2": w2_in, "w3": w3_in,
            "ident": ident,
        })
        ov_maps.append([nl for nl, _ in halves_ov])

    global LAST
    res = run_bass_kernel_spmd(
        nc, in_maps, core_ids=list(range(N_CORES)), trace=TRACE, **TRACE_KW
    )
    LAST = res

    out = x.copy()
    for c in range(N_CORES):
        r = res.results[c]
        # [128c, T, 8, 49] -> [H, 49, 128]
        o = np.asarray(r["outr"]).astype(np.float32).reshape(
            128, T, NPT, LFULL).transpose(1, 2, 3, 0).reshape(
            H, LFULL, 128)
        oc = o[:NODES_PER_CORE]
        for s, nl in enumerate(ov_maps[c]):
            oc[nl] += o[H_MAIN + s]
        out[c * NODES_PER_CORE:(c + 1) * NODES_PER_CORE] += oc
    return out
